# revision 21
# baseline (speedup 1.0000x reference)
"""Trainium2 Bass kernel for nn_BaselineTargetHead (per-sample dynamic MLP).

Strategy: data-parallel over 8 NeuronCores, 8 samples per core.
Per sample the chain is 5 per-sample linear layers over 64 spatial positions:
  [1024,2048] @ [2048,64] -> sigmoid -> ... -> [1,128] @ [128,64] + b

The kernel is HBM-bound (weights are used exactly once), so weights and the
layer-1 input travel as fp8 e3m4 (4 mantissa bits). Weights are pre-scaled by
64 on the host so they sit in e3m4's normal range; the 1/64 is folded into the
ScalarE activation's free affine (out = sigmoid(scale*psum + bias)).
Activations stay fp16, so layers 2-5 run mixed fp8-lhsT x fp16-rhs matmuls
(legal: only fp32 must be paired with fp32).

Device kernel (per core, per sample):
  - one packed per-sample fp8 slab [x | w5 | L1 | L2 | L3 | L4], with each
    layer's weights m-block-major so DMA chunk order == compute order. Four
    DMA chunks per sample (x+w5, L1 m0-3, L1 m4-7, L2-L4) on the sync ring,
    4-deep buffering so the ring never waits on buffer recycling.
  - matmul: lhsT = W^T tile [128(Cin), 128(Cout)] fp8, rhs = activation tile
    [128(Cin), 64(spatial)], accumulate over Cin tiles in PSUM fp32.
  - ScalarE applies scale+bias+sigmoid fused, writing fp16 activation tiles
    that feed the next layer without any transposition.
  - per-sample [1,64] results collect into one SBUF tile; single output DMA.
"""

import numpy as np
import ml_dtypes

import concourse.bass as bass
import concourse.mybir as mybir
import concourse.tile as tile
from concourse.bass_utils import run_bass_kernel_spmd

N_CORES = 8
B = 64
S_PER_CORE = B // N_CORES  # 8 samples per core
HW = 64  # 8x8 spatial positions
DIMS = [2048, 1024, 512, 256, 128, 1]
LAYERS = [(2048, 1024), (1024, 512), (512, 256), (256, 128)]  # (Cin, Cout) of fc1..fc4
W_SCALE_FP8 = 64.0  # lift weights into e3m4's normal range; undone in the act scale
FP8_CLIP = 15.0  # e3m4 saturates to inf above 15.5

X_COLS = (2048 // 128) * HW  # 1024
W5_COLS = 32  # w5 in col 0, zero-padded to 32 cols for a legal M=32 matmul
L_COLS = [(ci // 128) * co for ci, co in LAYERS]  # 16384, 4096, 1024, 256
# slab column map: [x | L1a (m0-3) | L1b (m4-7) | L2 | L3 | L4]
C0_END = X_COLS  # 1024
C1_END = C0_END + L_COLS[0] // 2  # 9216
C2_END = C1_END + L_COLS[0] // 2  # 17408
TOT_COLS = C2_END + L_COLS[1] + L_COLS[2] + L_COLS[3]  # 22784
L3_OFF = L_COLS[1]  # offset of L3 inside the C3 chunk
L4_OFF = L_COLS[1] + L_COLS[2]
# bias image columns per sample: fc1 m0..7 | fc2 m0..3 | fc3 m0..1 | fc4 m0 | fc5
BIAS_COL0 = [0, 8, 12, 14]
BIAS_COLS = 16

def _split_ctrl_multiwaits(nc):
    """walrus in this env rejects >1 sync-wait per instruction. Move extra
    waits onto NOPs placed immediately before, on the same engine — engines
    execute in order, so this is semantically identical."""
    n_fixed = 0
    for bb in nc.main_func.blocks:
        insts = bb.instructions
        i = 0
        while i < len(insts):
            ins = insts[i]
            si = ins.sync_info
            if si is not None and si.on_wait and len(si.on_wait) > 1:
                waits = list(si.on_wait)
                new_nops = []
                for j, w in enumerate(waits[1:]):
                    nop = mybir.InstNoOp(name=f"{ins.name}-splitw-{j}", ins=[], outs=[])
                    nop.engine = ins.engine
                    nop.sync_info = mybir.SyncInfo(on_update=[], on_wait=[w])
                    new_nops.append(nop)
                si.on_wait = [waits[0]]
                insts[i:i] = new_nops
                i += len(new_nops)
                n_fixed += 1
            i += 1
    return n_fixed


# per-sample DMA chunk boundaries (columns), in compute-consumption order.
# Few, large chunks: the tile framework round-robins HWDGE completions over 8
# sem lanes and serializes lane reuse, capping DMAs in flight at ~8 — bigger
# chunks keep more bytes buffered ahead of the PE.
CHUNKS = [
    ("A", 0, X_COLS + 2 * 2048),      # x + L1 m0-1   (640 KB)
    ("B", X_COLS + 2 * 2048, C2_END),  # L1 m2-7      (1.5 MB)
    ("C", C2_END, C2_END + L_COLS[1]),  # L2          (512 KB)
    ("D", C2_END + L_COLS[1], TOT_COLS),  # L3 + L4   (160 KB)
]
# absolute slab column of weight block (li, m, k)
_L_BASE = [X_COLS, C2_END, C2_END + L_COLS[1], C2_END + L_COLS[1] + L_COLS[2]]


def _wcol_abs(li, m, k):
    kt = LAYERS[li][0] // 128
    return _L_BASE[li] + (m * kt + k) * 128


def _build_nc():
    f8 = mybir.dt.float8e3
    f16 = mybir.dt.float16
    f32 = mybir.dt.float32
    nc = bass.Bass()
    slab_d = nc.dram_tensor("slab", [S_PER_CORE, 128, TOT_COLS], f8, kind="ExternalInput")
    # final-layer weights stay fp16: their quantization error hits the output
    # with no sigmoid attenuation (fp8 w5 alone costs ~2% rel err)
    w5_d = nc.dram_tensor("w5", [128, S_PER_CORE * W5_COLS], f16, kind="ExternalInput")
    bias_d = nc.dram_tensor("bias", [128, S_PER_CORE * BIAS_COLS], f32, kind="ExternalInput")
    out_d = nc.dram_tensor("out", [S_PER_CORE, HW], f32, kind="ExternalOutput")

    sig = mybir.ActivationFunctionType.Sigmoid
    ident = mybir.ActivationFunctionType.Identity
    inv_s = 1.0 / W_SCALE_FP8

    with tile.TileContext(nc) as tc:
        with (
            tc.tile_pool(name="wpool", bufs=1) as wpool,
            tc.tile_pool(name="qpool", bufs=2) as qpool,
            tc.tile_pool(name="misc", bufs=1) as misc,
            tc.tile_pool(name="psum", bufs=6, space="PSUM") as psum_pool,
        ):
            # small inputs on the ACT HWDGE ring so the SP ring carries
            # nothing but the per-sample slab stream
            bias_sb = misc.tile([128, S_PER_CORE * BIAS_COLS], f32)
            nc.scalar.dma_start(bias_sb[:], bias_d[:])
            w5_sb = misc.tile([128, S_PER_CORE * W5_COLS], f16)
            nc.scalar.dma_start(w5_sb[:], w5_d[:])
            collect = misc.tile([1, S_PER_CORE * HW], f32)

            # hoist the ~2.7us sigmoid ACT-table load into the DMA ramp-up
            # window via a throwaway 1-element sigmoid
            sig_warm = misc.tile([1, 1], f32, name="sig_warm")
            nc.vector.memset(sig_warm[:], 0.0)
            nc.scalar.activation(sig_warm[:], sig_warm[:], sig, scale=1.0)

            # whole slab is SBUF-resident: issue every chunk DMA up front in
            # consumption order on one HWDGE ring; no buffer recycling, so the
            # SDMA engines stream HBM continuously at full rate.
            ct = {}
            for s in range(S_PER_CORE):
                for cname, lo, hi in CHUNKS:
                    t = wpool.tile([128, hi - lo], f8, tag=f"{cname}{s}", name=f"{cname}{s}")
                    nc.sync.dma_start(t[:], slab_d[s, :, lo:hi])
                    ct[(cname, s)] = t

            def wsrc(s, li, m, k):
                # (tile, col) of weight block (li, m, k) of sample s
                c = _wcol_abs(li, m, k)
                for cname, lo, hi in CHUNKS:
                    if lo <= c < hi:
                        assert c + 128 <= hi, f"block straddles chunk: {li},{m},{k}"
                        return ct[(cname, s)], c - lo
                raise AssertionError("column out of range")

            # qs[s][li] = SBUF activation tile after layer li+1 of sample s
            qs = [[None] * 4 for _ in range(S_PER_CORE)]
            l1ps = {}  # (s, half) -> PSUM bank for L1 m-tiles 4h..4h+3
            l1done = {}

            def emit_mtile(s, li, m):
                """MM group for m-tile m of layer li of sample s.

                L1 (8 m-tiles, would be 8 narrow ACTIVATEs at ~420ns each)
                accumulates 4 m-tiles per PSUM bank; a whole-bank DVE
                bias-add (reads every slice -> orders after all PE writes;
                concurrent PE-write + DVE-read of one bank is a fatal HW
                collision) then ONE [128,256] sigmoid drains it.  The small
                layers keep per-m-tile ACTIVATEs with the bias carried in
                the ACT's per-partition bias operand — fine-grained so the
                next layer's k-MMs start as each m-block lands."""
                cin, cout = LAYERS[li]
                kt, mt = cin // 128, cout // 128
                if qs[s][li] is None:
                    qs[s][li] = qpool.tile(
                        [128, mt * HW], f16, tag=f"q{li}", name=f"q{li}_{s}"
                    )
                qn = qs[s][li]
                q_prev = ct[("A", s)][:, 0:X_COLS] if li == 0 else qs[s][li - 1][:]
                # L2 contracts halfB's q1 blocks first: halfB's wide sigmoid
                # fires mid-previous-slot (Y order below), halfA's only at the
                # slot boundary — k-order [4..7, 0..3] hides that latency
                korder = (
                    list(range(4, kt)) + list(range(4))
                    if li == 1
                    else range(kt)
                )
                if li == 0:
                    half = m // 4
                    if (s, half) not in l1ps:
                        l1ps[(s, half)] = psum_pool.tile(
                            [128, 4 * HW], f32, tag="psL1", bufs=4,
                            name=f"psL1_{s}_{half}",
                        )
                        l1done[(s, half)] = 0
                    ps = l1ps[(s, half)]
                    psm = ps[:, (m % 4) * HW : (m % 4 + 1) * HW]
                else:
                    ps = psum_pool.tile(
                        [128, HW], f32, tag="ps", bufs=4, name=f"ps{li}_{m}_{s}"
                    )
                    psm = ps[:]
                for ki, k in enumerate(korder):
                    wt, wcol = wsrc(s, li, m, k)
                    nc.tensor.matmul(
                        psm, wt[:, wcol : wcol + 128],
                        q_prev[:, k * HW : (k + 1) * HW],
                        start=(ki == 0), stop=(ki == kt - 1),
                    )
                if li == 0:
                    half = m // 4
                    l1done[(s, half)] += 1
                    if l1done[(s, half)] == 4:
                        ps3 = ps[:].rearrange("p (m w) -> p m w", w=HW)
                        c0 = s * BIAS_COLS + 4 * half
                        bb = (
                            bias_sb[:, c0 : c0 + 4]
                            .unsqueeze(-1)
                            .broadcast_to((128, 4, HW))
                        )
                        nc.vector.scalar_tensor_tensor(
                            ps3, ps3, 1.0, bb,
                            mybir.AluOpType.mult, mybir.AluOpType.add,
                        )
                        nc.scalar.activation(
                            qn[:, half * 4 * HW : (half + 1) * 4 * HW],
                            ps[:], sig, scale=inv_s,
                        )
                else:
                    bcol = s * BIAS_COLS + BIAS_COL0[li] + m
                    nc.scalar.activation(
                        qn[:, m * HW : (m + 1) * HW], ps[:], sig,
                        bias=bias_sb[:, bcol : bcol + 1], scale=inv_s,
                    )

            def emit_l5(s):
                ps5 = psum_pool.tile([128, HW], f32, tag="ps", bufs=4, name=f"ps5_{s}")
                nc.tensor.matmul(
                    ps5[0:32, :], w5_sb[:, s * W5_COLS : (s + 1) * W5_COLS],
                    qs[s][3][:, 0:HW], start=True, stop=True,
                )
                # bias-add on the idle DVE: an Identity ACTIVATE here would
                # force an ACT table-set swap (sigmoid<->identity) per sample
                # (~2.7us stall + a 16KB table DMA that jams SDMA engine 0)
                b5col = s * BIAS_COLS + 15
                nc.vector.tensor_scalar_add(
                    collect[0:1, s * HW : (s + 1) * HW], ps5[0:1, :],
                    bias_sb[0:1, b5col : b5col + 1],
                )

            def tail_units(s):
                # the ACT-latency-gated back-layers of sample s, as 8 units
                return (
                    [(s, 1, m) for m in range(4)]
                    + [(s, 2, m) for m in range(2)]
                    + [(s, 3, 0), (s, "L5", 0)]
                )

            def emit_unit(u):
                s, li, m = u
                if li == "L5":
                    emit_l5(s)
                else:
                    emit_mtile(s, li, m)

            # software pipeline: sample s+1's DMA-fed L1 m-tiles interleave
            # with sample s's ACT-latency-gated L2..L5 chain, so the PE never
            # sits in the ~0.5us sigmoid-wait bubbles (which also kept
            # re-tripping the HAM throttle).
            for m in range(8):
                emit_mtile(0, 0, m)
            # within a slot, emit the m-tile needing the next sample's chunk
            # B's final columns FIRST: the whole slot's DMA wait consolidates
            # into that one group, and the rest runs stall-free — scattered
            # ~1us data stalls otherwise keep re-tripping the HAM throttle.
            Y_ORDER = [7, 4, 5, 6, 0, 1, 2, 3]
            for s in range(S_PER_CORE - 1):
                units = tail_units(s)
                for i in range(8):
                    emit_mtile(s + 1, 0, Y_ORDER[i])
                    emit_unit(units[i])
                if s == S_PER_CORE - 3:
                    # samples 0..5 are done: ship their outputs early on the
                    # idle SP ring so only the last sliver rides the tail
                    nc.sync.dma_start(
                        out_d[0 : S_PER_CORE - 2, :],
                        collect[0:1, 0 : (S_PER_CORE - 2) * HW],
                    )
            for u in tail_units(S_PER_CORE - 1):
                emit_unit(u)
            nc.sync.dma_start(
                out_d[S_PER_CORE - 2 : S_PER_CORE, :],
                collect[0:1, (S_PER_CORE - 2) * HW : S_PER_CORE * HW],
            )

    _split_ctrl_multiwaits(nc)
    return nc


_NC_CACHE = None


def _get_nc():
    global _NC_CACHE
    if _NC_CACHE is None:
        _NC_CACHE = _build_nc()
    return _NC_CACHE


def _to_fp8(a):
    return np.clip(a, -FP8_CLIP, FP8_CLIP).astype(ml_dtypes.float8_e3m4)


def _prep_core(inputs, c):
    """Build the per-core input map (numpy only, host-side layout prep)."""
    sl = slice(c * S_PER_CORE, (c + 1) * S_PER_CORE)

    # x image: [S, 128, 1024] with img[s, p, k*64+h] = x[s, k*128+p, h]
    x = inputs["target_in_vec"][sl].reshape(S_PER_CORE, 2048 // 128, 128, HW)
    ximg = _to_fp8(x.transpose(0, 2, 1, 3).reshape(S_PER_CORE, 128, X_COLS))
    w5pad = np.zeros((S_PER_CORE, 128, W5_COLS), np.float16)
    w5pad[:, :, 0] = inputs["target_fc5w"][sl, 0, :, 0, 0]  # [S, 128]
    w5img = np.ascontiguousarray(
        w5pad.transpose(1, 0, 2).reshape(128, S_PER_CORE * W5_COLS)
    )

    # per-layer m-block-major weight images:
    # img[s, p, (m*kt+k)*128 + c] = w[s, m*128+c, k*128+p] * 64
    wparts = []
    for li, (cin, cout) in enumerate(LAYERS):
        kt, mt = cin // 128, cout // 128
        w = inputs[f"target_fc{li + 1}w"][sl, :, :, 0, 0]  # [S, Cout, Cin]
        wt = w.reshape(S_PER_CORE, mt, 128, kt, 128)  # [s, m, c, k, p]
        wt = wt.transpose(0, 4, 1, 3, 2).reshape(S_PER_CORE, 128, kt * mt * 128)
        wparts.append(_to_fp8(wt * W_SCALE_FP8))
    slab = np.ascontiguousarray(np.concatenate([ximg] + wparts, axis=2))
    assert slab.shape[2] == TOT_COLS

    bias = np.zeros((S_PER_CORE, 128, BIAS_COLS), np.float32)
    for li, (cin, cout) in enumerate(LAYERS):
        b = inputs[f"target_fc{li + 1}b"][sl]  # [S, Cout]
        # L1 bias is pre-scaled by 64: DVE adds it to the x64 PSUM
        # accumulator and the sigmoid's scale=1/64 undoes both together.
        # L2-L4 go through the ACT bias operand (applied after scale).
        scale = W_SCALE_FP8 if li == 0 else 1.0
        bias[:, :, BIAS_COL0[li] : BIAS_COL0[li] + cout // 128] = (
            scale * b.reshape(S_PER_CORE, cout // 128, 128).transpose(0, 2, 1)
        )
    bias[:, 0, 15] = inputs["target_fc5b"][sl, 0]
    bias = np.ascontiguousarray(bias.transpose(1, 0, 2).reshape(128, -1))

    return {"slab": slab, "w5": w5img, "bias": bias}


def kernel(**inputs):
    inputs = {k: np.asarray(v) for k, v in inputs.items()}
    nc = _get_nc()
    in_maps = [_prep_core(inputs, c) for c in range(N_CORES)]
    res = run_bass_kernel_spmd(nc, in_maps, list(range(N_CORES)))
    out = np.concatenate([np.asarray(res.results[c]["out"]) for c in range(N_CORES)], axis=0)
    return out.reshape(B, 8, 8).astype(np.float32)



# revision 25
# speedup vs baseline: 1.0223x; 1.0223x over previous
"""Trainium2 Bass kernel for nn_BaselineTargetHead (per-sample dynamic MLP).

Strategy: data-parallel over 8 NeuronCores, 8 samples per core.
Per sample the chain is 5 per-sample linear layers over 64 spatial positions:
  [1024,2048] @ [2048,64] -> sigmoid -> ... -> [1,128] @ [128,64] + b

The kernel is HBM-bound (weights are used exactly once), so weights and the
layer-1 input travel as fp8 e3m4 (4 mantissa bits). Weights are pre-scaled by
64 on the host so they sit in e3m4's normal range; the 1/64 is folded into the
ScalarE activation's free affine (out = sigmoid(scale*psum + bias)).
Activations stay fp16, so layers 2-5 run mixed fp8-lhsT x fp16-rhs matmuls
(legal: only fp32 must be paired with fp32).

Device kernel (per core, per sample):
  - one packed per-sample fp8 slab [x | w5 | L1 | L2 | L3 | L4], with each
    layer's weights m-block-major so DMA chunk order == compute order. Four
    DMA chunks per sample (x+w5, L1 m0-3, L1 m4-7, L2-L4) on the sync ring,
    4-deep buffering so the ring never waits on buffer recycling.
  - matmul: lhsT = W^T tile [128(Cin), 128(Cout)] fp8, rhs = activation tile
    [128(Cin), 64(spatial)], accumulate over Cin tiles in PSUM fp32.
  - ScalarE applies scale+bias+sigmoid fused, writing fp16 activation tiles
    that feed the next layer without any transposition.
  - per-sample [1,64] results collect into one SBUF tile; single output DMA.
"""

import numpy as np
import ml_dtypes

import concourse.bass as bass
import concourse.mybir as mybir
import concourse.tile as tile
from concourse.bass_utils import run_bass_kernel_spmd

N_CORES = 8
B = 64
S_PER_CORE = B // N_CORES  # 8 samples per core
HW = 64  # 8x8 spatial positions
DIMS = [2048, 1024, 512, 256, 128, 1]
LAYERS = [(2048, 1024), (1024, 512), (512, 256), (256, 128)]  # (Cin, Cout) of fc1..fc4
W_SCALE_FP8 = 64.0  # lift weights into e3m4's normal range; undone in the act scale
FP8_CLIP = 15.0  # e3m4 saturates to inf above 15.5

X_COLS = (2048 // 128) * HW  # 1024
W5_COLS = 32  # w5 in col 0, zero-padded to 32 cols for a legal M=32 matmul
L_COLS = [(ci // 128) * co for ci, co in LAYERS]  # 16384, 4096, 1024, 256
# slab column map: [x | L1a (m0-3) | L1b (m4-7) | L2 | L3 | L4]
C0_END = X_COLS  # 1024
C1_END = C0_END + L_COLS[0] // 2  # 9216
C2_END = C1_END + L_COLS[0] // 2  # 17408
TOT_COLS = C2_END + L_COLS[1] + L_COLS[2] + L_COLS[3]  # 22784
L3_OFF = L_COLS[1]  # offset of L3 inside the C3 chunk
L4_OFF = L_COLS[1] + L_COLS[2]
# bias image columns per sample: fc1 m0..7 | fc2 m0..3 | fc3 m0..1 | fc4 m0 | fc5
BIAS_COL0 = [0, 8, 12, 14]
BIAS_COLS = 16

def _split_ctrl_multiwaits(nc):
    """walrus in this env rejects >1 sync-wait per instruction. Move extra
    waits onto NOPs placed immediately before, on the same engine — engines
    execute in order, so this is semantically identical."""
    n_fixed = 0
    for bb in nc.main_func.blocks:
        insts = bb.instructions
        i = 0
        while i < len(insts):
            ins = insts[i]
            si = ins.sync_info
            if si is not None and si.on_wait and len(si.on_wait) > 1:
                waits = list(si.on_wait)
                new_nops = []
                for j, w in enumerate(waits[1:]):
                    nop = mybir.InstNoOp(name=f"{ins.name}-splitw-{j}", ins=[], outs=[])
                    nop.engine = ins.engine
                    nop.sync_info = mybir.SyncInfo(on_update=[], on_wait=[w])
                    new_nops.append(nop)
                si.on_wait = [waits[0]]
                insts[i:i] = new_nops
                i += len(new_nops)
                n_fixed += 1
            i += 1
    return n_fixed


# per-sample DMA chunk boundaries (columns), in compute-consumption order.
# Few, large chunks: the tile framework round-robins HWDGE completions over 8
# sem lanes and serializes lane reuse, capping DMAs in flight at ~8 — bigger
# chunks keep more bytes buffered ahead of the PE.
CHUNKS = [
    ("A", 0, X_COLS + 2 * 2048),      # x + L1 m0-1   (640 KB)
    ("B", X_COLS + 2 * 2048, C2_END),  # L1 m2-7      (1.5 MB)
    ("C", C2_END, C2_END + L_COLS[1]),  # L2          (512 KB)
    ("D", C2_END + L_COLS[1], TOT_COLS),  # L3 + L4   (160 KB)
]
# absolute slab column of weight block (li, m, k)
_L_BASE = [X_COLS, C2_END, C2_END + L_COLS[1], C2_END + L_COLS[1] + L_COLS[2]]


def _wcol_abs(li, m, k):
    kt = LAYERS[li][0] // 128
    return _L_BASE[li] + (m * kt + k) * 128


def _build_nc():
    f8 = mybir.dt.float8e3
    f16 = mybir.dt.float16
    f32 = mybir.dt.float32
    nc = bass.Bass()
    slab_d = nc.dram_tensor("slab", [S_PER_CORE, 128, TOT_COLS], f8, kind="ExternalInput")
    # final-layer weights stay fp16: their quantization error hits the output
    # with no sigmoid attenuation (fp8 w5 alone costs ~2% rel err)
    w5_d = nc.dram_tensor("w5", [128, S_PER_CORE * W5_COLS], f16, kind="ExternalInput")
    bias_d = nc.dram_tensor("bias", [128, S_PER_CORE * BIAS_COLS], f32, kind="ExternalInput")
    out_d = nc.dram_tensor("out", [S_PER_CORE, HW], f32, kind="ExternalOutput")

    sig = mybir.ActivationFunctionType.Sigmoid
    ident = mybir.ActivationFunctionType.Identity
    inv_s = 1.0 / W_SCALE_FP8

    with tile.TileContext(nc) as tc:
        with (
            tc.tile_pool(name="wpool", bufs=1) as wpool,
            tc.tile_pool(name="qpool", bufs=2) as qpool,
            tc.tile_pool(name="misc", bufs=1) as misc,
            tc.tile_pool(name="psum", bufs=6, space="PSUM") as psum_pool,
        ):
            # small inputs on the ACT HWDGE ring so the SP ring carries
            # nothing but the per-sample slab stream
            bias_sb = misc.tile([128, S_PER_CORE * BIAS_COLS], f32)
            nc.scalar.dma_start(bias_sb[:], bias_d[:])
            w5_sb = misc.tile([128, S_PER_CORE * W5_COLS], f16)
            nc.scalar.dma_start(w5_sb[:], w5_d[:])
            collect = misc.tile([1, S_PER_CORE * HW], f32)

            # hoist the ~2.7us sigmoid ACT-table load into the DMA ramp-up
            # window via a throwaway 1-element sigmoid
            sig_warm = misc.tile([1, 1], f32, name="sig_warm")
            nc.vector.memset(sig_warm[:], 0.0)
            nc.scalar.activation(sig_warm[:], sig_warm[:], sig, scale=1.0)

            # whole slab is SBUF-resident: issue every chunk DMA up front in
            # consumption order on one HWDGE ring; no buffer recycling, so the
            # SDMA engines stream HBM continuously at full rate.
            ct = {}
            for s in range(S_PER_CORE):
                for cname, lo, hi in CHUNKS:
                    t = wpool.tile([128, hi - lo], f8, tag=f"{cname}{s}", name=f"{cname}{s}")
                    nc.sync.dma_start(t[:], slab_d[s, :, lo:hi])
                    ct[(cname, s)] = t

            def wsrc(s, li, m, k):
                # (tile, col) of weight block (li, m, k) of sample s
                c = _wcol_abs(li, m, k)
                for cname, lo, hi in CHUNKS:
                    if lo <= c < hi:
                        assert c + 128 <= hi, f"block straddles chunk: {li},{m},{k}"
                        return ct[(cname, s)], c - lo
                raise AssertionError("column out of range")

            # qs[s][li] = SBUF activation tile after layer li+1 of sample s
            qs = [[None] * 4 for _ in range(S_PER_CORE)]

            # throwaway matmuls on resident data into a spare PSUM bank.
            # Placed in front of a data-gated group they convert PE idle
            # (which re-trips the HAM clock throttle after a ~3.4us window,
            # halving matmul speed) into harmless activity.
            warm_ps = psum_pool.tile([128, HW], f32, tag="warm", bufs=1)

            def emit_fill(n):
                a0 = ct[("A", 0)]
                for _ in range(n):
                    nc.tensor.matmul(
                        warm_ps[:], a0[:, 128:256], a0[:, 0:HW],
                        start=True, stop=True,
                    )

            def emit_mtile(s, li, m):
                """MM group for m-tile m of layer li of sample s.

                L1 (8 m-tiles, would be 8 narrow ACTIVATEs at ~420ns each)
                accumulates 4 m-tiles per PSUM bank; a whole-bank DVE
                bias-add (reads every slice -> orders after all PE writes;
                concurrent PE-write + DVE-read of one bank is a fatal HW
                collision) then ONE [128,256] sigmoid drains it.  The small
                layers keep per-m-tile ACTIVATEs with the bias carried in
                the ACT's per-partition bias operand — fine-grained so the
                next layer's k-MMs start as each m-block lands."""
                cin, cout = LAYERS[li]
                kt, mt = cin // 128, cout // 128
                if qs[s][li] is None:
                    qs[s][li] = qpool.tile(
                        [128, mt * HW], f16, tag=f"q{li}", name=f"q{li}_{s}"
                    )
                qn = qs[s][li]
                q_prev = ct[("A", s)][:, 0:X_COLS] if li == 0 else qs[s][li - 1][:]
                ps = psum_pool.tile(
                    [128, HW], f32, tag="ps", bufs=6, name=f"ps{li}_{m}_{s}"
                )
                for k in range(kt):
                    wt, wcol = wsrc(s, li, m, k)
                    nc.tensor.matmul(
                        ps[:], wt[:, wcol : wcol + 128],
                        q_prev[:, k * HW : (k + 1) * HW],
                        start=(k == 0), stop=(k == kt - 1),
                    )
                bcol = s * BIAS_COLS + BIAS_COL0[li] + m
                nc.scalar.activation(
                    qn[:, m * HW : (m + 1) * HW], ps[:], sig,
                    bias=bias_sb[:, bcol : bcol + 1], scale=inv_s,
                )

            def emit_l5(s):
                ps5 = psum_pool.tile([128, HW], f32, tag="ps", bufs=6, name=f"ps5_{s}")
                nc.tensor.matmul(
                    ps5[0:32, :], w5_sb[:, s * W5_COLS : (s + 1) * W5_COLS],
                    qs[s][3][:, 0:HW], start=True, stop=True,
                )
                # bias-add on the idle DVE: an Identity ACTIVATE here would
                # force an ACT table-set swap (sigmoid<->identity) per sample
                # (~2.7us stall + a 16KB table DMA that jams SDMA engine 0)
                b5col = s * BIAS_COLS + 15
                nc.vector.tensor_scalar_add(
                    collect[0:1, s * HW : (s + 1) * HW], ps5[0:1, :],
                    bias_sb[0:1, b5col : b5col + 1],
                )

            def tail_units(s):
                # the ACT-latency-gated back-layers of sample s, as 8 units
                return (
                    [(s, 1, m) for m in range(4)]
                    + [(s, 2, m) for m in range(2)]
                    + [(s, 3, 0), (s, "L5", 0)]
                )

            def emit_unit(u):
                s, li, m = u
                if li == "L5":
                    emit_l5(s)
                else:
                    emit_mtile(s, li, m)

            # software pipeline: sample s+1's DMA-fed L1 m-tiles interleave
            # with sample s's ACT-latency-gated L2..L5 chain, so the PE never
            # sits in the ~0.5us sigmoid-wait bubbles (which also kept
            # re-tripping the HAM throttle).
            # sample 0 ramps with the DMA stream: fill the pre-data window and
            # the inter-chunk waits so HAM warms up before real work
            emit_fill(100)
            for m in range(8):
                emit_mtile(0, 0, m)
                if m in (1, 2):  # about to wait on chunk B0 / mid-B0
                    emit_fill(25)
            # within a slot, emit the m-tile needing the next sample's chunk
            # B's final columns FIRST: the whole slot's DMA wait consolidates
            # into that one group (prefixed with fill so the wait isn't PE
            # idle), and the rest runs stall-free.
            Y_ORDER = [7, 0, 1, 2, 3, 4, 5, 6]
            for s in range(S_PER_CORE - 1):
                units = tail_units(s)
                for i in range(8):
                    if i == 0:
                        emit_fill(20)
                    emit_mtile(s + 1, 0, Y_ORDER[i])
                    emit_unit(units[i])
                if s == S_PER_CORE - 3:
                    # samples 0..5 are done: ship their outputs early on the
                    # idle SP ring so only the last sliver rides the tail
                    nc.sync.dma_start(
                        out_d[0 : S_PER_CORE - 2, :],
                        collect[0:1, 0 : (S_PER_CORE - 2) * HW],
                    )
            for u in tail_units(S_PER_CORE - 1):
                emit_unit(u)
            nc.sync.dma_start(
                out_d[S_PER_CORE - 2 : S_PER_CORE, :],
                collect[0:1, (S_PER_CORE - 2) * HW : S_PER_CORE * HW],
            )

    _split_ctrl_multiwaits(nc)
    return nc


_NC_CACHE = None


def _get_nc():
    global _NC_CACHE
    if _NC_CACHE is None:
        _NC_CACHE = _build_nc()
    return _NC_CACHE


def _to_fp8(a):
    return np.clip(a, -FP8_CLIP, FP8_CLIP).astype(ml_dtypes.float8_e3m4)


def _prep_core(inputs, c):
    """Build the per-core input map (numpy only, host-side layout prep)."""
    sl = slice(c * S_PER_CORE, (c + 1) * S_PER_CORE)

    # x image: [S, 128, 1024] with img[s, p, k*64+h] = x[s, k*128+p, h]
    x = inputs["target_in_vec"][sl].reshape(S_PER_CORE, 2048 // 128, 128, HW)
    ximg = _to_fp8(x.transpose(0, 2, 1, 3).reshape(S_PER_CORE, 128, X_COLS))
    w5pad = np.zeros((S_PER_CORE, 128, W5_COLS), np.float16)
    w5pad[:, :, 0] = inputs["target_fc5w"][sl, 0, :, 0, 0]  # [S, 128]
    w5img = np.ascontiguousarray(
        w5pad.transpose(1, 0, 2).reshape(128, S_PER_CORE * W5_COLS)
    )

    # per-layer m-block-major weight images:
    # img[s, p, (m*kt+k)*128 + c] = w[s, m*128+c, k*128+p] * 64
    wparts = []
    for li, (cin, cout) in enumerate(LAYERS):
        kt, mt = cin // 128, cout // 128
        w = inputs[f"target_fc{li + 1}w"][sl, :, :, 0, 0]  # [S, Cout, Cin]
        wt = w.reshape(S_PER_CORE, mt, 128, kt, 128)  # [s, m, c, k, p]
        wt = wt.transpose(0, 4, 1, 3, 2).reshape(S_PER_CORE, 128, kt * mt * 128)
        wparts.append(_to_fp8(wt * W_SCALE_FP8))
    slab = np.ascontiguousarray(np.concatenate([ximg] + wparts, axis=2))
    assert slab.shape[2] == TOT_COLS

    bias = np.zeros((S_PER_CORE, 128, BIAS_COLS), np.float32)
    for li, (cin, cout) in enumerate(LAYERS):
        b = inputs[f"target_fc{li + 1}b"][sl]  # [S, Cout]
        # L1 bias is pre-scaled by 64: DVE adds it to the x64 PSUM
        # accumulator and the sigmoid's scale=1/64 undoes both together.
        # L2-L4 go through the ACT bias operand (applied after scale).
        scale = W_SCALE_FP8 if li == 0 else 1.0
        bias[:, :, BIAS_COL0[li] : BIAS_COL0[li] + cout // 128] = (
            scale * b.reshape(S_PER_CORE, cout // 128, 128).transpose(0, 2, 1)
        )
    bias[:, 0, 15] = inputs["target_fc5b"][sl, 0]
    bias = np.ascontiguousarray(bias.transpose(1, 0, 2).reshape(128, -1))

    return {"slab": slab, "w5": w5img, "bias": bias}


def kernel(**inputs):
    inputs = {k: np.asarray(v) for k, v in inputs.items()}
    nc = _get_nc()
    in_maps = [_prep_core(inputs, c) for c in range(N_CORES)]
    res = run_bass_kernel_spmd(nc, in_maps, list(range(N_CORES)))
    out = np.concatenate([np.asarray(res.results[c]["out"]) for c in range(N_CORES)], axis=0)
    return out.reshape(B, 8, 8).astype(np.float32)



# revision 28
# speedup vs baseline: 1.0254x; 1.0030x over previous
"""Trainium2 Bass kernel for nn_BaselineTargetHead (per-sample dynamic MLP).

Strategy: data-parallel over 8 NeuronCores, 8 samples per core.
Per sample the chain is 5 per-sample linear layers over 64 spatial positions:
  [1024,2048] @ [2048,64] -> sigmoid -> ... -> [1,128] @ [128,64] + b

The kernel is HBM-bound (weights are used exactly once), so weights and the
layer-1 input travel as fp8 e3m4 (4 mantissa bits). Weights are pre-scaled by
64 on the host so they sit in e3m4's normal range; the 1/64 is folded into the
ScalarE activation's free affine (out = sigmoid(scale*psum + bias)).
Activations stay fp16, so layers 2-5 run mixed fp8-lhsT x fp16-rhs matmuls
(legal: only fp32 must be paired with fp32).

Device kernel (per core, per sample):
  - one packed per-sample fp8 slab [x | w5 | L1 | L2 | L3 | L4], with each
    layer's weights m-block-major so DMA chunk order == compute order. Four
    DMA chunks per sample (x+w5, L1 m0-3, L1 m4-7, L2-L4) on the sync ring,
    4-deep buffering so the ring never waits on buffer recycling.
  - matmul: lhsT = W^T tile [128(Cin), 128(Cout)] fp8, rhs = activation tile
    [128(Cin), 64(spatial)], accumulate over Cin tiles in PSUM fp32.
  - ScalarE applies scale+bias+sigmoid fused, writing fp16 activation tiles
    that feed the next layer without any transposition.
  - per-sample [1,64] results collect into one SBUF tile; single output DMA.
"""

import numpy as np
import ml_dtypes

import concourse.bass as bass
import concourse.mybir as mybir
import concourse.tile as tile
from concourse.bass_utils import run_bass_kernel_spmd

N_CORES = 8
B = 64
S_PER_CORE = B // N_CORES  # 8 samples per core
HW = 64  # 8x8 spatial positions
DIMS = [2048, 1024, 512, 256, 128, 1]
LAYERS = [(2048, 1024), (1024, 512), (512, 256), (256, 128)]  # (Cin, Cout) of fc1..fc4
W_SCALE_FP8 = 64.0  # lift weights into e3m4's normal range; undone in the act scale
FP8_CLIP = 15.0  # e3m4 saturates to inf above 15.5

X_COLS = (2048 // 128) * HW  # 1024
W5_COLS = 32  # w5 in col 0, zero-padded to 32 cols for a legal M=32 matmul
L_COLS = [(ci // 128) * co for ci, co in LAYERS]  # 16384, 4096, 1024, 256
# slab column map: [x | L1a (m0-3) | L1b (m4-7) | L2 | L3 | L4]
C0_END = X_COLS  # 1024
C1_END = C0_END + L_COLS[0] // 2  # 9216
C2_END = C1_END + L_COLS[0] // 2  # 17408
TOT_COLS = C2_END + L_COLS[1] + L_COLS[2] + L_COLS[3]  # 22784
L3_OFF = L_COLS[1]  # offset of L3 inside the C3 chunk
L4_OFF = L_COLS[1] + L_COLS[2]
# bias image columns per sample: fc1 m0..7 | fc2 m0..3 | fc3 m0..1 | fc4 m0 | fc5
BIAS_COL0 = [0, 8, 12, 14]
BIAS_COLS = 16

def _split_ctrl_multiwaits(nc):
    """walrus in this env rejects >1 sync-wait per instruction. Move extra
    waits onto NOPs placed immediately before, on the same engine — engines
    execute in order, so this is semantically identical."""
    n_fixed = 0
    for bb in nc.main_func.blocks:
        insts = bb.instructions
        i = 0
        while i < len(insts):
            ins = insts[i]
            si = ins.sync_info
            if si is not None and si.on_wait and len(si.on_wait) > 1:
                waits = list(si.on_wait)
                new_nops = []
                for j, w in enumerate(waits[1:]):
                    nop = mybir.InstNoOp(name=f"{ins.name}-splitw-{j}", ins=[], outs=[])
                    nop.engine = ins.engine
                    nop.sync_info = mybir.SyncInfo(on_update=[], on_wait=[w])
                    new_nops.append(nop)
                si.on_wait = [waits[0]]
                insts[i:i] = new_nops
                i += len(new_nops)
                n_fixed += 1
            i += 1
    return n_fixed


# per-sample DMA chunk boundaries (columns), in compute-consumption order.
# Few, large chunks: the tile framework round-robins HWDGE completions over 8
# sem lanes and serializes lane reuse, capping DMAs in flight at ~8 — bigger
# chunks keep more bytes buffered ahead of the PE.
CHUNKS = [
    ("A", 0, X_COLS + 2 * 2048),      # x + L1 m0-1   (640 KB)
    ("B", X_COLS + 2 * 2048, C2_END),  # L1 m2-7      (1.5 MB)
    ("C", C2_END, C2_END + L_COLS[1]),  # L2          (512 KB)
    ("D", C2_END + L_COLS[1], TOT_COLS),  # L3 + L4   (160 KB)
]
# absolute slab column of weight block (li, m, k)
_L_BASE = [X_COLS, C2_END, C2_END + L_COLS[1], C2_END + L_COLS[1] + L_COLS[2]]


def _wcol_abs(li, m, k):
    kt = LAYERS[li][0] // 128
    return _L_BASE[li] + (m * kt + k) * 128


def _build_nc():
    f8 = mybir.dt.float8e3
    f16 = mybir.dt.float16
    f32 = mybir.dt.float32
    nc = bass.Bass()
    slab_d = nc.dram_tensor("slab", [S_PER_CORE, 128, TOT_COLS], f8, kind="ExternalInput")
    # final-layer weights stay fp16: their quantization error hits the output
    # with no sigmoid attenuation (fp8 w5 alone costs ~2% rel err)
    w5_d = nc.dram_tensor("w5", [128, S_PER_CORE * W5_COLS], f16, kind="ExternalInput")
    bias_d = nc.dram_tensor("bias", [128, S_PER_CORE * BIAS_COLS], f32, kind="ExternalInput")
    out_d = nc.dram_tensor("out", [S_PER_CORE, HW], f32, kind="ExternalOutput")

    sig = mybir.ActivationFunctionType.Sigmoid
    ident = mybir.ActivationFunctionType.Identity
    inv_s = 1.0 / W_SCALE_FP8

    with tile.TileContext(nc) as tc:
        with (
            tc.tile_pool(name="wpool", bufs=1) as wpool,
            tc.tile_pool(name="qpool", bufs=2) as qpool,
            tc.tile_pool(name="misc", bufs=1) as misc,
            tc.tile_pool(name="psum", bufs=6, space="PSUM") as psum_pool,
        ):
            # small inputs on the ACT HWDGE ring so the SP ring carries
            # nothing but the per-sample slab stream
            bias_sb = misc.tile([128, S_PER_CORE * BIAS_COLS], f32)
            nc.scalar.dma_start(bias_sb[:], bias_d[:])
            w5_sb = misc.tile([128, S_PER_CORE * W5_COLS], f16)
            nc.scalar.dma_start(w5_sb[:], w5_d[:])
            collect = misc.tile([1, S_PER_CORE * HW], f32)

            # hoist the ~2.7us sigmoid ACT-table load into the DMA ramp-up
            # window via a throwaway 1-element sigmoid
            sig_warm = misc.tile([1, 1], f32, name="sig_warm")
            nc.vector.memset(sig_warm[:], 0.0)
            nc.scalar.activation(sig_warm[:], sig_warm[:], sig, scale=1.0)

            # whole slab is SBUF-resident: issue every chunk DMA up front in
            # consumption order on one HWDGE ring; no buffer recycling, so the
            # SDMA engines stream HBM continuously at full rate.
            ct = {}
            for s in range(S_PER_CORE):
                for cname, lo, hi in CHUNKS:
                    t = wpool.tile([128, hi - lo], f8, tag=f"{cname}{s}", name=f"{cname}{s}")
                    nc.sync.dma_start(t[:], slab_d[s, :, lo:hi])
                    ct[(cname, s)] = t

            def wsrc(s, li, m, k):
                # (tile, col) of weight block (li, m, k) of sample s
                c = _wcol_abs(li, m, k)
                for cname, lo, hi in CHUNKS:
                    if lo <= c < hi:
                        assert c + 128 <= hi, f"block straddles chunk: {li},{m},{k}"
                        return ct[(cname, s)], c - lo
                raise AssertionError("column out of range")

            # qs[s][li] = SBUF activation tile after layer li+1 of sample s
            qs = [[None] * 4 for _ in range(S_PER_CORE)]

            # throwaway matmuls on resident data into a spare PSUM bank.
            # Placed in front of a data-gated group they convert PE idle
            # (which re-trips the HAM clock throttle after a ~3.4us window,
            # halving matmul speed) into harmless activity.
            warm_ps = psum_pool.tile([128, HW], f32, tag="warm", bufs=1)

            def emit_fill(n):
                a0 = ct[("A", 0)]
                for _ in range(n):
                    nc.tensor.matmul(
                        warm_ps[:], a0[:, 128:256], a0[:, 0:HW],
                        start=True, stop=True,
                    )

            def emit_mtile(s, li, m):
                """MM group for m-tile m of layer li of sample s.

                L1 (8 m-tiles, would be 8 narrow ACTIVATEs at ~420ns each)
                accumulates 4 m-tiles per PSUM bank; a whole-bank DVE
                bias-add (reads every slice -> orders after all PE writes;
                concurrent PE-write + DVE-read of one bank is a fatal HW
                collision) then ONE [128,256] sigmoid drains it.  The small
                layers keep per-m-tile ACTIVATEs with the bias carried in
                the ACT's per-partition bias operand — fine-grained so the
                next layer's k-MMs start as each m-block lands."""
                cin, cout = LAYERS[li]
                kt, mt = cin // 128, cout // 128
                if qs[s][li] is None:
                    qs[s][li] = qpool.tile(
                        [128, mt * HW], f16, tag=f"q{li}", name=f"q{li}_{s}"
                    )
                qn = qs[s][li]
                q_prev = ct[("A", s)][:, 0:X_COLS] if li == 0 else qs[s][li - 1][:]
                ps = psum_pool.tile(
                    [128, HW], f32, tag="ps", bufs=6, name=f"ps{li}_{m}_{s}"
                )
                for k in range(kt):
                    wt, wcol = wsrc(s, li, m, k)
                    nc.tensor.matmul(
                        ps[:], wt[:, wcol : wcol + 128],
                        q_prev[:, k * HW : (k + 1) * HW],
                        start=(k == 0), stop=(k == kt - 1),
                    )
                bcol = s * BIAS_COLS + BIAS_COL0[li] + m
                nc.scalar.activation(
                    qn[:, m * HW : (m + 1) * HW], ps[:], sig,
                    bias=bias_sb[:, bcol : bcol + 1], scale=inv_s,
                )

            def emit_l5(s):
                ps5 = psum_pool.tile([128, HW], f32, tag="ps", bufs=6, name=f"ps5_{s}")
                nc.tensor.matmul(
                    ps5[0:32, :], w5_sb[:, s * W5_COLS : (s + 1) * W5_COLS],
                    qs[s][3][:, 0:HW], start=True, stop=True,
                )
                # bias-add on the idle DVE: an Identity ACTIVATE here would
                # force an ACT table-set swap (sigmoid<->identity) per sample
                # (~2.7us stall + a 16KB table DMA that jams SDMA engine 0)
                b5col = s * BIAS_COLS + 15
                nc.vector.tensor_scalar_add(
                    collect[0:1, s * HW : (s + 1) * HW], ps5[0:1, :],
                    bias_sb[0:1, b5col : b5col + 1],
                )

            def tail_units(s):
                # the ACT-latency-gated back-layers of sample s, as 8 units
                return (
                    [(s, 1, m) for m in range(4)]
                    + [(s, 2, m) for m in range(2)]
                    + [(s, 3, 0), (s, "L5", 0)]
                )

            def emit_unit(u):
                s, li, m = u
                if li == "L5":
                    emit_l5(s)
                else:
                    emit_mtile(s, li, m)

            # software pipeline: sample s+1's DMA-fed L1 m-tiles interleave
            # with sample s's ACT-latency-gated L2..L5 chain, so the PE never
            # sits in the ~0.5us sigmoid-wait bubbles (which also kept
            # re-tripping the HAM throttle).
            # sample 0 ramps with the DMA stream: fill the pre-data window and
            # the inter-chunk waits so HAM warms up before real work
            emit_fill(55)
            for m in range(8):
                emit_mtile(0, 0, m)
                if m in (1, 2):  # about to wait on chunk B0 / mid-B0
                    emit_fill(20)
            # within a slot, emit the m-tile needing the next sample's chunk
            # B's final columns FIRST: the whole slot's DMA wait consolidates
            # into that one group (prefixed with fill so the wait isn't PE
            # idle), and the rest runs stall-free.
            Y_ORDER = [7, 0, 1, 2, 3, 4, 5, 6]
            for s in range(S_PER_CORE - 1):
                units = tail_units(s)
                for i in range(8):
                    if i == 0:
                        emit_fill(10)
                    emit_mtile(s + 1, 0, Y_ORDER[i])
                    emit_unit(units[i])
                if s == S_PER_CORE - 3:
                    # samples 0..5 are done: ship their outputs early on the
                    # idle SP ring so only the last sliver rides the tail
                    nc.sync.dma_start(
                        out_d[0 : S_PER_CORE - 2, :],
                        collect[0:1, 0 : (S_PER_CORE - 2) * HW],
                    )
            for u in tail_units(S_PER_CORE - 1):
                emit_unit(u)
            nc.sync.dma_start(
                out_d[S_PER_CORE - 2 : S_PER_CORE, :],
                collect[0:1, (S_PER_CORE - 2) * HW : S_PER_CORE * HW],
            )

    _split_ctrl_multiwaits(nc)
    return nc


_NC_CACHE = None


def _get_nc():
    global _NC_CACHE
    if _NC_CACHE is None:
        _NC_CACHE = _build_nc()
    return _NC_CACHE


def _to_fp8(a):
    return np.clip(a, -FP8_CLIP, FP8_CLIP).astype(ml_dtypes.float8_e3m4)


def _prep_core(inputs, c):
    """Build the per-core input map (numpy only, host-side layout prep)."""
    sl = slice(c * S_PER_CORE, (c + 1) * S_PER_CORE)

    # x image: [S, 128, 1024] with img[s, p, k*64+h] = x[s, k*128+p, h]
    x = inputs["target_in_vec"][sl].reshape(S_PER_CORE, 2048 // 128, 128, HW)
    ximg = _to_fp8(x.transpose(0, 2, 1, 3).reshape(S_PER_CORE, 128, X_COLS))
    w5pad = np.zeros((S_PER_CORE, 128, W5_COLS), np.float16)
    w5pad[:, :, 0] = inputs["target_fc5w"][sl, 0, :, 0, 0]  # [S, 128]
    w5img = np.ascontiguousarray(
        w5pad.transpose(1, 0, 2).reshape(128, S_PER_CORE * W5_COLS)
    )

    # per-layer m-block-major weight images:
    # img[s, p, (m*kt+k)*128 + c] = w[s, m*128+c, k*128+p] * 64
    wparts = []
    for li, (cin, cout) in enumerate(LAYERS):
        kt, mt = cin // 128, cout // 128
        w = inputs[f"target_fc{li + 1}w"][sl, :, :, 0, 0]  # [S, Cout, Cin]
        wt = w.reshape(S_PER_CORE, mt, 128, kt, 128)  # [s, m, c, k, p]
        wt = wt.transpose(0, 4, 1, 3, 2).reshape(S_PER_CORE, 128, kt * mt * 128)
        wparts.append(_to_fp8(wt * W_SCALE_FP8))
    slab = np.ascontiguousarray(np.concatenate([ximg] + wparts, axis=2))
    assert slab.shape[2] == TOT_COLS

    bias = np.zeros((S_PER_CORE, 128, BIAS_COLS), np.float32)
    for li, (cin, cout) in enumerate(LAYERS):
        b = inputs[f"target_fc{li + 1}b"][sl]  # [S, Cout]
        bias[:, :, BIAS_COL0[li] : BIAS_COL0[li] + cout // 128] = b.reshape(
            S_PER_CORE, cout // 128, 128
        ).transpose(0, 2, 1)
    bias[:, 0, 15] = inputs["target_fc5b"][sl, 0]
    bias = np.ascontiguousarray(bias.transpose(1, 0, 2).reshape(128, -1))

    return {"slab": slab, "w5": w5img, "bias": bias}


def kernel(**inputs):
    inputs = {k: np.asarray(v) for k, v in inputs.items()}
    nc = _get_nc()
    in_maps = [_prep_core(inputs, c) for c in range(N_CORES)]
    res = run_bass_kernel_spmd(nc, in_maps, list(range(N_CORES)))
    out = np.concatenate([np.asarray(res.results[c]["out"]) for c in range(N_CORES)], axis=0)
    return out.reshape(B, 8, 8).astype(np.float32)



# revision 29
# speedup vs baseline: 1.0617x; 1.0355x over previous
"""Trainium2 Bass kernel for nn_BaselineTargetHead (per-sample dynamic MLP).

Strategy: data-parallel over 8 NeuronCores, 8 samples per core.
Per sample the chain is 5 per-sample linear layers over 64 spatial positions:
  [1024,2048] @ [2048,64] -> sigmoid -> ... -> [1,128] @ [128,64] + b

The kernel is HBM-bound (weights are used exactly once), so weights and the
layer-1 input travel as fp8 e3m4 (4 mantissa bits). Weights are pre-scaled by
64 on the host so they sit in e3m4's normal range; the 1/64 is folded into the
ScalarE activation's free affine (out = sigmoid(scale*psum + bias)).
Activations stay fp16, so layers 2-5 run mixed fp8-lhsT x fp16-rhs matmuls
(legal: only fp32 must be paired with fp32).

Device kernel (per core, per sample):
  - one packed per-sample fp8 slab [x | w5 | L1 | L2 | L3 | L4], with each
    layer's weights m-block-major so DMA chunk order == compute order. Four
    DMA chunks per sample (x+w5, L1 m0-3, L1 m4-7, L2-L4) on the sync ring,
    4-deep buffering so the ring never waits on buffer recycling.
  - matmul: lhsT = W^T tile [128(Cin), 128(Cout)] fp8, rhs = activation tile
    [128(Cin), 64(spatial)], accumulate over Cin tiles in PSUM fp32.
  - ScalarE applies scale+bias+sigmoid fused, writing fp16 activation tiles
    that feed the next layer without any transposition.
  - per-sample [1,64] results collect into one SBUF tile; single output DMA.
"""

import numpy as np
import ml_dtypes

import concourse.bass as bass
import concourse.mybir as mybir
import concourse.tile as tile
from concourse.bass_utils import run_bass_kernel_spmd

N_CORES = 8
B = 64
S_PER_CORE = B // N_CORES  # 8 samples per core
HW = 64  # 8x8 spatial positions
DIMS = [2048, 1024, 512, 256, 128, 1]
LAYERS = [(2048, 1024), (1024, 512), (512, 256), (256, 128)]  # (Cin, Cout) of fc1..fc4
W_SCALE_FP8 = 64.0  # lift weights into e3m4's normal range; undone in the act scale
FP8_CLIP = 15.0  # e3m4 saturates to inf above 15.5

X_COLS = (2048 // 128) * HW  # 1024
W5_COLS = 32  # w5 in col 0, zero-padded to 32 cols for a legal M=32 matmul
L_COLS = [(ci // 128) * co for ci, co in LAYERS]  # 16384, 4096, 1024, 256
# slab column map: [x | L1a (m0-3) | L1b (m4-7) | L2 | L3 | L4]
C0_END = X_COLS  # 1024
C1_END = C0_END + L_COLS[0] // 2  # 9216
C2_END = C1_END + L_COLS[0] // 2  # 17408
TOT_COLS = C2_END + L_COLS[1] + L_COLS[2] + L_COLS[3]  # 22784
L3_OFF = L_COLS[1]  # offset of L3 inside the C3 chunk
L4_OFF = L_COLS[1] + L_COLS[2]
# bias image columns per sample: fc1 m0..7 | fc2 m0..3 | fc3 m0..1 | fc4 m0 | fc5
BIAS_COL0 = [0, 8, 12, 14]
BIAS_COLS = 16

def _split_ctrl_multiwaits(nc):
    """walrus in this env rejects >1 sync-wait per instruction. Move extra
    waits onto NOPs placed immediately before, on the same engine — engines
    execute in order, so this is semantically identical."""
    n_fixed = 0
    for bb in nc.main_func.blocks:
        insts = bb.instructions
        i = 0
        while i < len(insts):
            ins = insts[i]
            si = ins.sync_info
            if si is not None and si.on_wait and len(si.on_wait) > 1:
                waits = list(si.on_wait)
                new_nops = []
                for j, w in enumerate(waits[1:]):
                    nop = mybir.InstNoOp(name=f"{ins.name}-splitw-{j}", ins=[], outs=[])
                    nop.engine = ins.engine
                    nop.sync_info = mybir.SyncInfo(on_update=[], on_wait=[w])
                    new_nops.append(nop)
                si.on_wait = [waits[0]]
                insts[i:i] = new_nops
                i += len(new_nops)
                n_fixed += 1
            i += 1
    return n_fixed


# per-sample DMA chunk boundaries (columns), in compute-consumption order.
# Few, large chunks: the tile framework round-robins HWDGE completions over 8
# sem lanes and serializes lane reuse, capping DMAs in flight at ~8 — bigger
# chunks keep more bytes buffered ahead of the PE.
CHUNKS = [
    ("A", 0, X_COLS + 2 * 2048),      # x + L1 m0-1   (640 KB)
    ("B", X_COLS + 2 * 2048, C2_END),  # L1 m2-7      (1.5 MB)
    ("C", C2_END, C2_END + L_COLS[1]),  # L2          (512 KB)
    ("D", C2_END + L_COLS[1], TOT_COLS),  # L3 + L4   (160 KB)
]
# absolute slab column of weight block (li, m, k)
_L_BASE = [X_COLS, C2_END, C2_END + L_COLS[1], C2_END + L_COLS[1] + L_COLS[2]]


def _wcol_abs(li, m, k):
    kt = LAYERS[li][0] // 128
    return _L_BASE[li] + (m * kt + k) * 128


def _build_nc():
    f8 = mybir.dt.float8e3
    f16 = mybir.dt.float16
    f32 = mybir.dt.float32
    nc = bass.Bass()
    slab_d = nc.dram_tensor("slab", [S_PER_CORE, 128, TOT_COLS], f8, kind="ExternalInput")
    # final-layer weights stay fp16: their quantization error hits the output
    # with no sigmoid attenuation (fp8 w5 alone costs ~2% rel err)
    w5_d = nc.dram_tensor("w5", [128, S_PER_CORE * W5_COLS], f16, kind="ExternalInput")
    bias_d = nc.dram_tensor("bias", [128, S_PER_CORE * BIAS_COLS], f32, kind="ExternalInput")
    out_d = nc.dram_tensor("out", [S_PER_CORE, HW], f32, kind="ExternalOutput")

    sig = mybir.ActivationFunctionType.Sigmoid
    ident = mybir.ActivationFunctionType.Identity
    inv_s = 1.0 / W_SCALE_FP8

    with tile.TileContext(nc) as tc:
        with (
            tc.tile_pool(name="wpool", bufs=1) as wpool,
            tc.tile_pool(name="qpool", bufs=2) as qpool,
            tc.tile_pool(name="misc", bufs=1) as misc,
            tc.tile_pool(name="psum", bufs=6, space="PSUM") as psum_pool,
        ):
            # small inputs on the ACT HWDGE ring so the SP ring carries
            # nothing but the per-sample slab stream
            bias_sb = misc.tile([128, S_PER_CORE * BIAS_COLS], f32)
            nc.scalar.dma_start(bias_sb[:], bias_d[:])
            w5_sb = misc.tile([128, S_PER_CORE * W5_COLS], f16)
            nc.scalar.dma_start(w5_sb[:], w5_d[:])
            collect = misc.tile([1, S_PER_CORE * HW], f32)

            # hoist the ~2.7us sigmoid ACT-table load into the DMA ramp-up
            # window via a throwaway 1-element sigmoid
            sig_warm = misc.tile([1, 1], f32, name="sig_warm")
            nc.vector.memset(sig_warm[:], 0.0)
            nc.scalar.activation(sig_warm[:], sig_warm[:], sig, scale=1.0)

            # whole slab is SBUF-resident: issue every chunk DMA up front in
            # consumption order on one HWDGE ring; no buffer recycling, so the
            # SDMA engines stream HBM continuously at full rate.
            ct = {}
            for s in range(S_PER_CORE):
                for cname, lo, hi in CHUNKS:
                    t = wpool.tile([128, hi - lo], f8, tag=f"{cname}{s}", name=f"{cname}{s}")
                    nc.sync.dma_start(t[:], slab_d[s, :, lo:hi])
                    ct[(cname, s)] = t

            def wsrc(s, li, m, k):
                # (tile, col) of weight block (li, m, k) of sample s
                c = _wcol_abs(li, m, k)
                for cname, lo, hi in CHUNKS:
                    if lo <= c < hi:
                        assert c + 128 <= hi, f"block straddles chunk: {li},{m},{k}"
                        return ct[(cname, s)], c - lo
                raise AssertionError("column out of range")

            # qs[s][li] = SBUF activation tile after layer li+1 of sample s
            qs = [[None] * 4 for _ in range(S_PER_CORE)]

            # throwaway matmuls on resident data into a spare PSUM bank.
            # Placed in front of a data-gated group they convert PE idle
            # (which re-trips the HAM clock throttle after a ~3.4us window,
            # halving matmul speed) into harmless activity.
            warm_ps = psum_pool.tile([128, HW], f32, tag="warm", bufs=1)

            def emit_fill(n):
                a0 = ct[("A", 0)]
                for _ in range(n):
                    nc.tensor.matmul(
                        warm_ps[:], a0[:, 128:256], a0[:, 0:HW],
                        start=True, stop=True,
                    )

            def emit_mtile(s, li, m):
                """MM group for m-tile m of layer li of sample s.

                L1 (8 m-tiles, would be 8 narrow ACTIVATEs at ~420ns each)
                accumulates 4 m-tiles per PSUM bank; a whole-bank DVE
                bias-add (reads every slice -> orders after all PE writes;
                concurrent PE-write + DVE-read of one bank is a fatal HW
                collision) then ONE [128,256] sigmoid drains it.  The small
                layers keep per-m-tile ACTIVATEs with the bias carried in
                the ACT's per-partition bias operand — fine-grained so the
                next layer's k-MMs start as each m-block lands."""
                cin, cout = LAYERS[li]
                kt, mt = cin // 128, cout // 128
                if qs[s][li] is None:
                    qs[s][li] = qpool.tile(
                        [128, mt * HW], f16, tag=f"q{li}", name=f"q{li}_{s}"
                    )
                qn = qs[s][li]
                q_prev = ct[("A", s)][:, 0:X_COLS] if li == 0 else qs[s][li - 1][:]
                ps = psum_pool.tile(
                    [128, HW], f32, tag="ps", bufs=7, name=f"ps{li}_{m}_{s}"
                )
                for k in range(kt):
                    wt, wcol = wsrc(s, li, m, k)
                    nc.tensor.matmul(
                        ps[:], wt[:, wcol : wcol + 128],
                        q_prev[:, k * HW : (k + 1) * HW],
                        start=(k == 0), stop=(k == kt - 1),
                    )
                bcol = s * BIAS_COLS + BIAS_COL0[li] + m
                nc.scalar.activation(
                    qn[:, m * HW : (m + 1) * HW], ps[:], sig,
                    bias=bias_sb[:, bcol : bcol + 1], scale=inv_s,
                )

            def emit_l5(s):
                ps5 = psum_pool.tile([128, HW], f32, tag="ps", bufs=7, name=f"ps5_{s}")
                nc.tensor.matmul(
                    ps5[0:32, :], w5_sb[:, s * W5_COLS : (s + 1) * W5_COLS],
                    qs[s][3][:, 0:HW], start=True, stop=True,
                )
                # bias-add on the idle DVE: an Identity ACTIVATE here would
                # force an ACT table-set swap (sigmoid<->identity) per sample
                # (~2.7us stall + a 16KB table DMA that jams SDMA engine 0)
                b5col = s * BIAS_COLS + 15
                nc.vector.tensor_scalar_add(
                    collect[0:1, s * HW : (s + 1) * HW], ps5[0:1, :],
                    bias_sb[0:1, b5col : b5col + 1],
                )

            def tail_units(s):
                # the ACT-latency-gated back-layers of sample s, as 8 units
                return (
                    [(s, 1, m) for m in range(4)]
                    + [(s, 2, m) for m in range(2)]
                    + [(s, 3, 0), (s, "L5", 0)]
                )

            def emit_unit(u):
                s, li, m = u
                if li == "L5":
                    emit_l5(s)
                else:
                    emit_mtile(s, li, m)

            # software pipeline: sample s+1's DMA-fed L1 m-tiles interleave
            # with sample s's ACT-latency-gated L2..L5 chain, so the PE never
            # sits in the ~0.5us sigmoid-wait bubbles (which also kept
            # re-tripping the HAM throttle).
            # sample 0 ramps with the DMA stream: fill the pre-data window and
            # the inter-chunk waits so HAM warms up before real work
            emit_fill(55)
            for m in range(8):
                emit_mtile(0, 0, m)
                if m in (1, 2):  # about to wait on chunk B0 / mid-B0
                    emit_fill(12)
            # within a slot, emit the m-tile needing the next sample's chunk
            # B's final columns FIRST: the whole slot's DMA wait consolidates
            # into that one group (prefixed with fill so the wait isn't PE
            # idle), and the rest runs stall-free.
            Y_ORDER = [7, 0, 1, 2, 3, 4, 5, 6]
            for s in range(S_PER_CORE - 1):
                units = tail_units(s)
                for i in range(8):
                    emit_mtile(s + 1, 0, Y_ORDER[i])
                    emit_unit(units[i])
                if s == S_PER_CORE - 3:
                    # samples 0..5 are done: ship their outputs early on the
                    # idle SP ring so only the last sliver rides the tail
                    nc.sync.dma_start(
                        out_d[0 : S_PER_CORE - 2, :],
                        collect[0:1, 0 : (S_PER_CORE - 2) * HW],
                    )
            for u in tail_units(S_PER_CORE - 1):
                emit_unit(u)
            nc.sync.dma_start(
                out_d[S_PER_CORE - 2 : S_PER_CORE, :],
                collect[0:1, (S_PER_CORE - 2) * HW : S_PER_CORE * HW],
            )

    _split_ctrl_multiwaits(nc)
    return nc


_NC_CACHE = None


def _get_nc():
    global _NC_CACHE
    if _NC_CACHE is None:
        _NC_CACHE = _build_nc()
    return _NC_CACHE


def _to_fp8(a):
    return np.clip(a, -FP8_CLIP, FP8_CLIP).astype(ml_dtypes.float8_e3m4)


def _prep_core(inputs, c):
    """Build the per-core input map (numpy only, host-side layout prep)."""
    sl = slice(c * S_PER_CORE, (c + 1) * S_PER_CORE)

    # x image: [S, 128, 1024] with img[s, p, k*64+h] = x[s, k*128+p, h]
    x = inputs["target_in_vec"][sl].reshape(S_PER_CORE, 2048 // 128, 128, HW)
    ximg = _to_fp8(x.transpose(0, 2, 1, 3).reshape(S_PER_CORE, 128, X_COLS))
    w5pad = np.zeros((S_PER_CORE, 128, W5_COLS), np.float16)
    w5pad[:, :, 0] = inputs["target_fc5w"][sl, 0, :, 0, 0]  # [S, 128]
    w5img = np.ascontiguousarray(
        w5pad.transpose(1, 0, 2).reshape(128, S_PER_CORE * W5_COLS)
    )

    # per-layer m-block-major weight images:
    # img[s, p, (m*kt+k)*128 + c] = w[s, m*128+c, k*128+p] * 64
    wparts = []
    for li, (cin, cout) in enumerate(LAYERS):
        kt, mt = cin // 128, cout // 128
        w = inputs[f"target_fc{li + 1}w"][sl, :, :, 0, 0]  # [S, Cout, Cin]
        wt = w.reshape(S_PER_CORE, mt, 128, kt, 128)  # [s, m, c, k, p]
        wt = wt.transpose(0, 4, 1, 3, 2).reshape(S_PER_CORE, 128, kt * mt * 128)
        wparts.append(_to_fp8(wt * W_SCALE_FP8))
    slab = np.ascontiguousarray(np.concatenate([ximg] + wparts, axis=2))
    assert slab.shape[2] == TOT_COLS

    bias = np.zeros((S_PER_CORE, 128, BIAS_COLS), np.float32)
    for li, (cin, cout) in enumerate(LAYERS):
        b = inputs[f"target_fc{li + 1}b"][sl]  # [S, Cout]
        bias[:, :, BIAS_COL0[li] : BIAS_COL0[li] + cout // 128] = b.reshape(
            S_PER_CORE, cout // 128, 128
        ).transpose(0, 2, 1)
    bias[:, 0, 15] = inputs["target_fc5b"][sl, 0]
    bias = np.ascontiguousarray(bias.transpose(1, 0, 2).reshape(128, -1))

    return {"slab": slab, "w5": w5img, "bias": bias}


def kernel(**inputs):
    inputs = {k: np.asarray(v) for k, v in inputs.items()}
    nc = _get_nc()
    in_maps = [_prep_core(inputs, c) for c in range(N_CORES)]
    res = run_bass_kernel_spmd(nc, in_maps, list(range(N_CORES)))
    out = np.concatenate([np.asarray(res.results[c]["out"]) for c in range(N_CORES)], axis=0)
    return out.reshape(B, 8, 8).astype(np.float32)



# revision 31
# speedup vs baseline: 1.0641x; 1.0023x over previous
"""Trainium2 Bass kernel for nn_BaselineTargetHead (per-sample dynamic MLP).

Strategy: data-parallel over 8 NeuronCores, 8 samples per core.
Per sample the chain is 5 per-sample linear layers over 64 spatial positions:
  [1024,2048] @ [2048,64] -> sigmoid -> ... -> [1,128] @ [128,64] + b

The kernel is HBM-bound (weights are used exactly once), so weights and the
layer-1 input travel as fp8 e3m4 (4 mantissa bits). Weights are pre-scaled by
64 on the host so they sit in e3m4's normal range; the 1/64 is folded into the
ScalarE activation's free affine (out = sigmoid(scale*psum + bias)).
Activations stay fp16, so layers 2-5 run mixed fp8-lhsT x fp16-rhs matmuls
(legal: only fp32 must be paired with fp32).

Device kernel (per core, per sample):
  - one packed per-sample fp8 slab [x | w5 | L1 | L2 | L3 | L4], with each
    layer's weights m-block-major so DMA chunk order == compute order. Four
    DMA chunks per sample (x+w5, L1 m0-3, L1 m4-7, L2-L4) on the sync ring,
    4-deep buffering so the ring never waits on buffer recycling.
  - matmul: lhsT = W^T tile [128(Cin), 128(Cout)] fp8, rhs = activation tile
    [128(Cin), 64(spatial)], accumulate over Cin tiles in PSUM fp32.
  - ScalarE applies scale+bias+sigmoid fused, writing fp16 activation tiles
    that feed the next layer without any transposition.
  - per-sample [1,64] results collect into one SBUF tile; single output DMA.
"""

import numpy as np
import ml_dtypes

import concourse.bass as bass
import concourse.mybir as mybir
import concourse.tile as tile
from concourse.bass_utils import run_bass_kernel_spmd

N_CORES = 8
B = 64
S_PER_CORE = B // N_CORES  # 8 samples per core
HW = 64  # 8x8 spatial positions
DIMS = [2048, 1024, 512, 256, 128, 1]
LAYERS = [(2048, 1024), (1024, 512), (512, 256), (256, 128)]  # (Cin, Cout) of fc1..fc4
W_SCALE_FP8 = 64.0  # lift weights into e3m4's normal range; undone in the act scale
FP8_CLIP = 15.0  # e3m4 saturates to inf above 15.5

X_COLS = (2048 // 128) * HW  # 1024
W5_COLS = 32  # w5 in col 0, zero-padded to 32 cols for a legal M=32 matmul
L_COLS = [(ci // 128) * co for ci, co in LAYERS]  # 16384, 4096, 1024, 256
# slab column map: [x | L1a (m0-3) | L1b (m4-7) | L2 | L3 | L4]
C0_END = X_COLS  # 1024
C1_END = C0_END + L_COLS[0] // 2  # 9216
C2_END = C1_END + L_COLS[0] // 2  # 17408
TOT_COLS = C2_END + L_COLS[1] + L_COLS[2] + L_COLS[3]  # 22784
L3_OFF = L_COLS[1]  # offset of L3 inside the C3 chunk
L4_OFF = L_COLS[1] + L_COLS[2]
# bias image columns per sample: fc1 m0..7 | fc2 m0..3 | fc3 m0..1 | fc4 m0 | fc5
BIAS_COL0 = [0, 8, 12, 14]
BIAS_COLS = 16

def _split_ctrl_multiwaits(nc):
    """walrus in this env rejects >1 sync-wait per instruction. Move extra
    waits onto NOPs placed immediately before, on the same engine — engines
    execute in order, so this is semantically identical."""
    n_fixed = 0
    for bb in nc.main_func.blocks:
        insts = bb.instructions
        i = 0
        while i < len(insts):
            ins = insts[i]
            si = ins.sync_info
            if si is not None and si.on_wait and len(si.on_wait) > 1:
                waits = list(si.on_wait)
                new_nops = []
                for j, w in enumerate(waits[1:]):
                    nop = mybir.InstNoOp(name=f"{ins.name}-splitw-{j}", ins=[], outs=[])
                    nop.engine = ins.engine
                    nop.sync_info = mybir.SyncInfo(on_update=[], on_wait=[w])
                    new_nops.append(nop)
                si.on_wait = [waits[0]]
                insts[i:i] = new_nops
                i += len(new_nops)
                n_fixed += 1
            i += 1
    return n_fixed


# per-sample DMA chunk boundaries (columns), in compute-consumption order.
# Few, large chunks: the tile framework round-robins HWDGE completions over 8
# sem lanes and serializes lane reuse, capping DMAs in flight at ~8 — bigger
# chunks keep more bytes buffered ahead of the PE.
CHUNKS = [
    ("A", 0, X_COLS + 2 * 2048),      # x + L1 m0-1   (640 KB)
    ("B", X_COLS + 2 * 2048, C2_END),  # L1 m2-7      (1.5 MB)
    ("C", C2_END, C2_END + L_COLS[1]),  # L2          (512 KB)
    ("D", C2_END + L_COLS[1], TOT_COLS),  # L3 + L4   (160 KB)
]
# absolute slab column of weight block (li, m, k)
_L_BASE = [X_COLS, C2_END, C2_END + L_COLS[1], C2_END + L_COLS[1] + L_COLS[2]]


def _wcol_abs(li, m, k):
    kt = LAYERS[li][0] // 128
    return _L_BASE[li] + (m * kt + k) * 128


def _build_nc():
    f8 = mybir.dt.float8e3
    f16 = mybir.dt.float16
    f32 = mybir.dt.float32
    nc = bass.Bass()
    slab_d = nc.dram_tensor("slab", [S_PER_CORE, 128, TOT_COLS], f8, kind="ExternalInput")
    # final-layer weights stay fp16: their quantization error hits the output
    # with no sigmoid attenuation (fp8 w5 alone costs ~2% rel err)
    w5_d = nc.dram_tensor("w5", [128, S_PER_CORE * W5_COLS], f16, kind="ExternalInput")
    bias_d = nc.dram_tensor("bias", [128, S_PER_CORE * BIAS_COLS], f32, kind="ExternalInput")
    out_d = nc.dram_tensor("out", [S_PER_CORE, HW], f32, kind="ExternalOutput")

    sig = mybir.ActivationFunctionType.Sigmoid
    ident = mybir.ActivationFunctionType.Identity
    inv_s = 1.0 / W_SCALE_FP8

    with tile.TileContext(nc) as tc:
        with (
            tc.tile_pool(name="wpool", bufs=1) as wpool,
            tc.tile_pool(name="qpool", bufs=2) as qpool,
            tc.tile_pool(name="misc", bufs=1) as misc,
            tc.tile_pool(name="psum", bufs=6, space="PSUM") as psum_pool,
        ):
            # small inputs on the ACT HWDGE ring so the SP ring carries
            # nothing but the per-sample slab stream
            bias_sb = misc.tile([128, S_PER_CORE * BIAS_COLS], f32)
            nc.scalar.dma_start(bias_sb[:], bias_d[:])
            w5_sb = misc.tile([128, S_PER_CORE * W5_COLS], f16)
            nc.scalar.dma_start(w5_sb[:], w5_d[:])
            collect = misc.tile([1, S_PER_CORE * HW], f32)

            # hoist the ~2.7us sigmoid ACT-table load into the DMA ramp-up
            # window via a throwaway 1-element sigmoid
            sig_warm = misc.tile([1, 1], f32, name="sig_warm")
            nc.vector.memset(sig_warm[:], 0.0)
            nc.scalar.activation(sig_warm[:], sig_warm[:], sig, scale=1.0)

            # whole slab is SBUF-resident: issue every chunk DMA up front in
            # consumption order on one HWDGE ring; no buffer recycling, so the
            # SDMA engines stream HBM continuously at full rate.
            # chunk stream order: consumption order, except the last two
            # samples' back-layer chunks swap (C7,D7 before C6,D6) — the two
            # straggler SDMA engines run ~11us behind the pack by stream end,
            # and this lets both samples' L2-L5 chains interleave at the tail
            # instead of serializing behind the very last bytes.
            S_LAST, S_PREV = S_PER_CORE - 1, S_PER_CORE - 2
            order = []
            for s in range(S_PREV):
                order += [(c, s) for c, _, _ in CHUNKS]
            order += [("A", S_PREV), ("B", S_PREV), ("A", S_LAST), ("B", S_LAST),
                      ("C", S_LAST), ("D", S_LAST), ("C", S_PREV), ("D", S_PREV)]
            cmap = {c: (lo, hi) for c, lo, hi in CHUNKS}
            ct = {}
            for cname, s in order:
                lo, hi = cmap[cname]
                t = wpool.tile([128, hi - lo], f8, tag=f"{cname}{s}", name=f"{cname}{s}")
                nc.sync.dma_start(t[:], slab_d[s, :, lo:hi])
                ct[(cname, s)] = t

            def wsrc(s, li, m, k):
                # (tile, col) of weight block (li, m, k) of sample s
                c = _wcol_abs(li, m, k)
                for cname, lo, hi in CHUNKS:
                    if lo <= c < hi:
                        assert c + 128 <= hi, f"block straddles chunk: {li},{m},{k}"
                        return ct[(cname, s)], c - lo
                raise AssertionError("column out of range")

            # qs[s][li] = SBUF activation tile after layer li+1 of sample s
            qs = [[None] * 4 for _ in range(S_PER_CORE)]

            # throwaway matmuls on resident data into a spare PSUM bank.
            # Placed in front of a data-gated group they convert PE idle
            # (which re-trips the HAM clock throttle after a ~3.4us window,
            # halving matmul speed) into harmless activity.
            warm_ps = psum_pool.tile([128, HW], f32, tag="warm", bufs=1)

            def emit_fill(n):
                a0 = ct[("A", 0)]
                for _ in range(n):
                    nc.tensor.matmul(
                        warm_ps[:], a0[:, 128:256], a0[:, 0:HW],
                        start=True, stop=True,
                    )

            def emit_mtile(s, li, m):
                """MM group for m-tile m of layer li of sample s.

                L1 (8 m-tiles, would be 8 narrow ACTIVATEs at ~420ns each)
                accumulates 4 m-tiles per PSUM bank; a whole-bank DVE
                bias-add (reads every slice -> orders after all PE writes;
                concurrent PE-write + DVE-read of one bank is a fatal HW
                collision) then ONE [128,256] sigmoid drains it.  The small
                layers keep per-m-tile ACTIVATEs with the bias carried in
                the ACT's per-partition bias operand — fine-grained so the
                next layer's k-MMs start as each m-block lands."""
                cin, cout = LAYERS[li]
                kt, mt = cin // 128, cout // 128
                if qs[s][li] is None:
                    qs[s][li] = qpool.tile(
                        [128, mt * HW], f16, tag=f"q{li}", name=f"q{li}_{s}"
                    )
                qn = qs[s][li]
                q_prev = ct[("A", s)][:, 0:X_COLS] if li == 0 else qs[s][li - 1][:]
                ps = psum_pool.tile(
                    [128, HW], f32, tag="ps", bufs=7, name=f"ps{li}_{m}_{s}"
                )
                for k in range(kt):
                    wt, wcol = wsrc(s, li, m, k)
                    nc.tensor.matmul(
                        ps[:], wt[:, wcol : wcol + 128],
                        q_prev[:, k * HW : (k + 1) * HW],
                        start=(k == 0), stop=(k == kt - 1),
                    )
                bcol = s * BIAS_COLS + BIAS_COL0[li] + m
                nc.scalar.activation(
                    qn[:, m * HW : (m + 1) * HW], ps[:], sig,
                    bias=bias_sb[:, bcol : bcol + 1], scale=inv_s,
                )

            def emit_l5(s):
                ps5 = psum_pool.tile([128, HW], f32, tag="ps", bufs=7, name=f"ps5_{s}")
                nc.tensor.matmul(
                    ps5[0:32, :], w5_sb[:, s * W5_COLS : (s + 1) * W5_COLS],
                    qs[s][3][:, 0:HW], start=True, stop=True,
                )
                # bias-add on the idle DVE: an Identity ACTIVATE here would
                # force an ACT table-set swap (sigmoid<->identity) per sample
                # (~2.7us stall + a 16KB table DMA that jams SDMA engine 0)
                b5col = s * BIAS_COLS + 15
                nc.vector.tensor_scalar_add(
                    collect[0:1, s * HW : (s + 1) * HW], ps5[0:1, :],
                    bias_sb[0:1, b5col : b5col + 1],
                )

            def tail_units(s):
                # the ACT-latency-gated back-layers of sample s, as 8 units
                return (
                    [(s, 1, m) for m in range(4)]
                    + [(s, 2, m) for m in range(2)]
                    + [(s, 3, 0), (s, "L5", 0)]
                )

            def emit_unit(u):
                s, li, m = u
                if li == "L5":
                    emit_l5(s)
                else:
                    emit_mtile(s, li, m)

            # software pipeline: sample s+1's DMA-fed L1 m-tiles interleave
            # with sample s's ACT-latency-gated L2..L5 chain, so the PE never
            # sits in the ~0.5us sigmoid-wait bubbles (which also kept
            # re-tripping the HAM throttle).
            # sample 0 ramps with the DMA stream: fill the pre-data window and
            # the inter-chunk waits so HAM warms up before real work
            emit_fill(55)
            for m in range(8):
                emit_mtile(0, 0, m)
                if m in (1, 2):  # about to wait on chunk B0 / mid-B0
                    emit_fill(12)
            # within a slot, emit the m-tile needing the next sample's chunk
            # B's final columns FIRST: the whole slot's DMA wait consolidates
            # into that one group (prefixed with fill so the wait isn't PE
            # idle), and the rest runs stall-free.
            Y_ORDER = [7, 0, 1, 2, 3, 4, 5, 6]
            for s in range(S_PER_CORE - 2):
                units = tail_units(s)
                for i in range(8):
                    emit_mtile(s + 1, 0, Y_ORDER[i])
                    emit_unit(units[i])
                if s == S_PER_CORE - 3:
                    # samples 0..5 are done: ship their outputs early on the
                    # idle SP ring so only the last sliver rides the tail
                    nc.sync.dma_start(
                        out_d[0 : S_PER_CORE - 2, :],
                        collect[0:1, 0 : (S_PER_CORE - 2) * HW],
                    )
            # final slot: L1 of the last sample only; then the last TWO
            # samples' ACT-latency-gated back layers interleave, each chain
            # hiding the other's sigmoid waits while the straggler DMA
            # engines deliver their final chunks.
            for i in range(8):
                emit_mtile(S_LAST, 0, Y_ORDER[i])
            for u7, u6 in zip(tail_units(S_LAST), tail_units(S_PREV)):
                emit_unit(u7)
                emit_unit(u6)
            nc.sync.dma_start(
                out_d[S_PER_CORE - 2 : S_PER_CORE, :],
                collect[0:1, (S_PER_CORE - 2) * HW : S_PER_CORE * HW],
            )

    _split_ctrl_multiwaits(nc)
    return nc


_NC_CACHE = None


def _get_nc():
    global _NC_CACHE
    if _NC_CACHE is None:
        _NC_CACHE = _build_nc()
    return _NC_CACHE


def _to_fp8(a):
    return np.clip(a, -FP8_CLIP, FP8_CLIP).astype(ml_dtypes.float8_e3m4)


def _prep_core(inputs, c):
    """Build the per-core input map (numpy only, host-side layout prep)."""
    sl = slice(c * S_PER_CORE, (c + 1) * S_PER_CORE)

    # x image: [S, 128, 1024] with img[s, p, k*64+h] = x[s, k*128+p, h]
    x = inputs["target_in_vec"][sl].reshape(S_PER_CORE, 2048 // 128, 128, HW)
    ximg = _to_fp8(x.transpose(0, 2, 1, 3).reshape(S_PER_CORE, 128, X_COLS))
    w5pad = np.zeros((S_PER_CORE, 128, W5_COLS), np.float16)
    w5pad[:, :, 0] = inputs["target_fc5w"][sl, 0, :, 0, 0]  # [S, 128]
    w5img = np.ascontiguousarray(
        w5pad.transpose(1, 0, 2).reshape(128, S_PER_CORE * W5_COLS)
    )

    # per-layer m-block-major weight images:
    # img[s, p, (m*kt+k)*128 + c] = w[s, m*128+c, k*128+p] * 64
    wparts = []
    for li, (cin, cout) in enumerate(LAYERS):
        kt, mt = cin // 128, cout // 128
        w = inputs[f"target_fc{li + 1}w"][sl, :, :, 0, 0]  # [S, Cout, Cin]
        wt = w.reshape(S_PER_CORE, mt, 128, kt, 128)  # [s, m, c, k, p]
        wt = wt.transpose(0, 4, 1, 3, 2).reshape(S_PER_CORE, 128, kt * mt * 128)
        wparts.append(_to_fp8(wt * W_SCALE_FP8))
    slab = np.ascontiguousarray(np.concatenate([ximg] + wparts, axis=2))
    assert slab.shape[2] == TOT_COLS

    bias = np.zeros((S_PER_CORE, 128, BIAS_COLS), np.float32)
    for li, (cin, cout) in enumerate(LAYERS):
        b = inputs[f"target_fc{li + 1}b"][sl]  # [S, Cout]
        bias[:, :, BIAS_COL0[li] : BIAS_COL0[li] + cout // 128] = b.reshape(
            S_PER_CORE, cout // 128, 128
        ).transpose(0, 2, 1)
    bias[:, 0, 15] = inputs["target_fc5b"][sl, 0]
    bias = np.ascontiguousarray(bias.transpose(1, 0, 2).reshape(128, -1))

    return {"slab": slab, "w5": w5img, "bias": bias}


def kernel(**inputs):
    inputs = {k: np.asarray(v) for k, v in inputs.items()}
    nc = _get_nc()
    in_maps = [_prep_core(inputs, c) for c in range(N_CORES)]
    res = run_bass_kernel_spmd(nc, in_maps, list(range(N_CORES)))
    out = np.concatenate([np.asarray(res.results[c]["out"]) for c in range(N_CORES)], axis=0)
    return out.reshape(B, 8, 8).astype(np.float32)

